# revision 26
# baseline (speedup 1.0000x reference)
"""Trainium2 Bass kernel (v5) for the nn_Coref span scorer (T=20000, widths 1..10).

Sharding: token axis split 8 ways (2500 owned span-starts per core, 2560-token
slab covers the width-10 halo); small MLP weights replicated (packed host-side
into 4 DMA-able tensors).

Per-core structure (chunk-pipelined: P0 P1 W0 P2 W1 P3 W2 P4 W3 W4):
  - preamble chunks (512 cols): per-token features via PE matmuls:
    ha/hb/e = attention MLP (e = exp(logit)), A = states@W1a, B = states@W1b,
    F = (embeds@W1c)*e (evac fused with the *e multiply), ebc = e broadcast
    to 75 partitions. Evac engines chosen per chunk so the DVE-idle lead-in
    chunks use DVE/Act and steady-state chunks use Act.
  - width phases (512..1024 cols, region-hazard pipelined against the
    preamble): per width n: running sums Z (Pool) and Cp (DVE) by shifted
    adds; t = relu(Z*(A+B_sh) + Cp)  [gauge trick: t = Z*h1, relu commutes
    with the positive Z scale]; W2 runs TRANSPOSED on the PE (lhsT = t block
    [75k x 128 spans], rhs = W2 half [75, 150] -> out [128 spans, 150] psum,
    150-cycle matmuls) and is evac'd with fused relu (Act).
  - r = relu(W2^T t) = Z*h2 ships to DRAM in bf16; the host applies the w3
    contraction (random-sign f32 dot, ~0.4% noise), rebuilds Z from the
    shipped e row by f64 cumsum, and computes score = w3.r/Z + b3.
"""
import sys

sys.path.insert(0, "/opt/trn_rl_repo")

import numpy as np
import ml_dtypes

BF16 = ml_dtypes.bfloat16

T = 20000
NCORES = 8
L = 2500          # owned span starts per core
SLAB = 2560       # token slab (covers spans + width-10 halo)
PADW = 16
EXT = SLAB + PADW
NW = 10
SD = 400          # state dim
ED = 300          # embed dim
HID = 150
HH = 75           # hidden half (plane size)
CH = 512          # preamble chunk
NCH = SLAB // CH
WCH = 1280        # width-phase column granule
NWP = SLAB // WCH  # 2 width phases
NBW = WCH // 128   # span blocks per width phase (10)

_CACHE = {}


def _build_program(use_b2, use_b1):
    from contextlib import ExitStack
    import concourse.bacc as bacc
    import concourse.tile as tile
    from concourse import mybir

    f32 = mybir.dt.float32
    b16 = mybir.dt.bfloat16
    AF = mybir.ActivationFunctionType
    ALU = mybir.AluOpType

    nc = bacc.Bacc("TRN2", target_bir_lowering=False, debug=False)

    d_sT = nc.dram_tensor("sT", [SD, SLAB], b16, kind="ExternalInput")
    d_eT = nc.dram_tensor("eT", [ED, SLAB], b16, kind="ExternalInput")
    # packed weights (host layout, see _prep_inputs):
    # pk128 [128, 1650]: aw1 k0..2 | w1a k0..2 | w1b k0..2 | w1c k0..1  (11 x 150)
    # pk75  [75, 752]:   aw2 h0,h1 | w2 h0,h1 (4 x 150) | aw3 h0,h1 (2 x 1) | b2 row0 [150]
    # pktail[44, 600]:   w1c k2 (44 rows) | aw1 k3 | w1a k3 | w1b k3 (16 rows)
    # pkb   [75, 7] f32: ab1 h0,h1 | ab2 h0,h1 | sb1 h0,h1 | ab3 at [0,6]
    d_pk128 = nc.dram_tensor("pk128", [128, 1650], b16, kind="ExternalInput")
    d_pk75 = nc.dram_tensor("pk75", [HH, 752], b16, kind="ExternalInput")
    d_pktail = nc.dram_tensor("pktail", [44, 600], b16, kind="ExternalInput")
    d_pkb = nc.dram_tensor("pkb", [HH, 7], f32, kind="ExternalInput")

    # r_out layout: [width_phase, width, span_in_block, block*150] (host dots w3)
    d_ro = nc.dram_tensor("r_out", [NW, 128, (SLAB // 128) * HID], b16,
                          kind="ExternalOutput")
    d_eo = nc.dram_tensor("e_out", [1, SLAB], b16, kind="ExternalOutput")

    with tile.TileContext(nc) as tc, ExitStack() as ctx:
        wp = ctx.enter_context(tc.tile_pool(name="wp", bufs=1))
        bp = ctx.enter_context(tc.tile_pool(name="bp", bufs=1))
        hp = ctx.enter_context(tc.tile_pool(name="hp", bufs=1))
        kp = ctx.enter_context(tc.tile_pool(name="kp", bufs=3))
        pp = ctx.enter_context(tc.tile_pool(name="pp", bufs=3, space="PSUM"))
        pw = ctx.enter_context(tc.tile_pool(name="pw", bufs=5, space="PSUM"))

        # ---- packed weights: 4 DMAs, then column views ----
        pk128 = wp.tile([128, 1650], b16, name="pk128", tag="pk128")
        pk75 = wp.tile([HH, 752], b16, name="pk75", tag="pk75")
        pktail = wp.tile([44, 600], b16, name="pktail", tag="pktail")
        pkb = wp.tile([HH, 7], f32, name="pkb", tag="pkb")
        nc.sync.dma_start(pk128[:, :], d_pk128[:, :])
        nc.sync.dma_start(pktail[:, :], d_pktail[:, :])
        nc.sync.dma_start(pkb[:, :], d_pkb[:, :])

        def col(tile_, i, w=HID):
            return tile_[:, i * w:(i + 1) * w]

        aw1 = [col(pk128, 0), col(pk128, 1), col(pk128, 2), pktail[0:16, 150:300]]
        w1a = [col(pk128, 3), col(pk128, 4), col(pk128, 5), pktail[0:16, 300:450]]
        w1b = [col(pk128, 6), col(pk128, 7), col(pk128, 8), pktail[0:16, 450:600]]
        w1c = [col(pk128, 9), col(pk128, 10), pktail[0:44, 0:150]]
        aw2 = [col(pk75, 0), col(pk75, 1)]
        w2h = [col(pk75, 2), col(pk75, 3)]   # W2 halves [75, 150]
        aw3 = [pk75[:, 600:601], pk75[:, 601:602]]
        b2r = pk75[0:1, 602:752]  # [1, 150] b2 row
        ab1 = [pkb[:, 0:1], pkb[:, 1:2]]
        ab2 = [pkb[:, 2:3], pkb[:, 3:4]]
        sb1 = [pkb[:, 4:5], pkb[:, 5:6]]
        ab3 = pkb[0:1, 6:7]

        # ---- input tiles; 3 column-group DMAs so chunk 0/1 start early ----
        SD_CH = [(0, 128), (128, 256), (256, 384), (384, 400)]
        ED_CH = [(0, 128), (128, 256), (256, 300)]
        sT = [hp.tile([k1 - k0, SLAB], b16, name=f"sT{i}", tag=f"sT{i}")
              for i, (k0, k1) in enumerate(SD_CH)]
        eT = [hp.tile([k1 - k0, SLAB], b16, name=f"eT{i}", tag=f"eT{i}")
              for i, (k0, k1) in enumerate(ED_CH)]
        first = True
        for c0, c1 in ((0, 512), (512, 1024), (1024, SLAB)):
            for i, (k0, k1) in enumerate(SD_CH):
                nc.sync.dma_start(sT[i][:, c0:c1], d_sT[k0:k1, c0:c1])
            if first:
                nc.sync.dma_start(pk75[:, :], d_pk75[:, :])
                first = False
            for i, (k0, k1) in enumerate(ED_CH):
                nc.sync.dma_start(eT[i][:, c0:c1], d_eT[k0:k1, c0:c1])

        # ---- persistent per-token feature tiles ----
        A3 = bp.tile([HH, 2, EXT], b16, name="A3", tag="A3")
        B3 = bp.tile([HH, 2, EXT], b16, name="B3", tag="B3")
        F3 = bp.tile([HH, 2, EXT], b16, name="F3", tag="F3")
        e16 = bp.tile([1, EXT], b16, name="e16", tag="e16")
        ebc = bp.tile([HH, EXT], b16, name="ebc", tag="ebc")
        nc.vector.memset(B3[:, :, SLAB:EXT], 0.0)
        nc.vector.memset(F3[:, :, SLAB:EXT], 0.0)
        nc.vector.memset(ebc[:, SLAB:EXT], 0.0)
        ha = hp.tile([HH, 2, SLAB], b16, name="ha", tag="ha")
        hb = hp.tile([HH, 2, SLAB], b16, name="hb", tag="hb")

        def mm_group(ps, wts, rhs_tiles, cs, m0, m1):
            for ki in range(len(wts)):
                nc.tensor.matmul(ps[:, :], wts[ki][:, m0:m1], rhs_tiles[ki][:, cs],
                                 start=(ki == 0), stop=(ki == len(wts) - 1))

        def preamble(c, lead=False):
            # lead=True: runs in the DVE-idle lead-in; evacs go to DVE
            cs = slice(c * CH, (c + 1) * CH)
            fmul = nc.gpsimd.tensor_mul

            def cpy(dst, src_):
                if lead:
                    nc.vector.tensor_copy(dst, src_)
                else:
                    nc.scalar.activation(dst, src_, AF.Copy)
            for m, (m0, m1) in enumerate(((0, HH), (HH, HID))):
                ps = pp.tile([HH, CH], f32, name=f"ps_ha{m}", tag="ps75")
                mm_group(ps, aw1, sT, cs, m0, m1)
                nc.scalar.activation(ha[:, m, cs], ps[:, :], AF.Relu, bias=ab1[m])
                ps = pp.tile([HH, CH], f32, name=f"ps_A{m}", tag="ps75")
                mm_group(ps, w1a, sT, cs, m0, m1)
                nc.scalar.activation(A3[:, m, cs], ps[:, :], AF.Copy)
            for m, (m0, m1) in enumerate(((0, HH), (HH, HID))):
                ps = pp.tile([HH, CH], f32, name=f"ps_hb{m}", tag="ps75")
                for ki in range(2):
                    nc.tensor.matmul(ps[:, :], aw2[ki][:, m0:m1], ha[:, ki, cs],
                                     start=(ki == 0), stop=(ki == 1))
                nc.scalar.activation(hb[:, m, cs], ps[:, :], AF.Relu, bias=ab2[m])
                ps = pp.tile([HH, CH], f32, name=f"ps_B{m}", tag="ps75")
                mm_group(ps, w1b, sT, cs, m0, m1)
                cpy(B3[:, m, cs], ps[:, :])
            ps = pp.tile([1, CH], f32, name="ps_e", tag="ps75")
            for ki in range(2):
                nc.tensor.matmul(ps[0:1, :], aw3[ki][:, 0:1], hb[:, ki, cs],
                                 start=(ki == 0), stop=(ki == 1))
            nc.scalar.activation(e16[0:1, cs], ps[0:1, :], AF.Exp, bias=ab3[0:1, :])
            nc.gpsimd.partition_broadcast(ebc[:, cs], e16[0:1, cs])
            for m, (m0, m1) in enumerate(((0, HH), (HH, HID))):
                ps = pp.tile([HH, CH], f32, name=f"ps_F{m}", tag="ps75")
                mm_group(ps, w1c, eT, cs, m0, m1)
                if use_b1:
                    bias = sb1[m]
                    nc.scalar.activation(F3[:, m, cs], ps[:, :], AF.Identity,
                                         bias=bias)
                    fmul(F3[:, m, cs], F3[:, m, cs], ebc[:, cs])
                elif lead:
                    # DVE may read PSUM; fuse the evacuation with *e
                    nc.vector.tensor_mul(F3[:, m, cs], ps[:, :], ebc[:, cs])
                else:
                    # Pool cannot access PSUM: Act-copy out, multiply in SBUF
                    nc.scalar.activation(F3[:, m, cs], ps[:, :], AF.Copy)
                    nc.gpsimd.tensor_mul(F3[:, m, cs], F3[:, m, cs], ebc[:, cs])

        def widthphase(w0, w1):
            W = w1 - w0
            nb = W // 128
            b0 = w0 // 128
            cs = slice(w0, w1)
            zb_prev = None
            cp_prev = None
            for n in range(1, NW + 1):
                sh = n - 1
                ss = slice(w0 + sh, w1 + sh)
                # Z accumulate (Pool)
                if n == 1:
                    zb = ebc[:, cs]
                else:
                    zbn = kp.tile([HH, W], b16, name="zb", tag=f"zb{w0}", bufs=3)
                    nc.gpsimd.tensor_add(zbn[:, :], zb_prev, ebc[:, ss])
                    zb = zbn[:, :]
                zb_prev = zb
                # Cp accumulate (DVE)
                if n == 1:
                    cp = F3[:, :, cs]
                else:
                    cpn = kp.tile([HH, 2, W], b16, name="cp", tag=f"cp{w0}", bufs=3)
                    nc.vector.tensor_add(cpn[:, :, :], cp_prev, F3[:, :, ss])
                    cp = cpn[:, :, :]
                cp_prev = cp
                # t = (A + B_sh) * Z + Cp, relu
                t3 = kp.tile([HH, 2, W], b16, name="t3", tag=f"t3{w0}", bufs=3)
                nc.gpsimd.tensor_add(t3[:, 0, :], A3[:, 0, cs], B3[:, 0, ss])
                nc.vector.tensor_add(t3[:, 1, :], A3[:, 1, cs], B3[:, 1, ss])
                zbv = zb.unsqueeze(1).broadcast_to([HH, 2, W])
                nc.gpsimd.tensor_mul(t3[:, :, :], t3[:, :, :], zbv)
                nc.vector.tensor_add(t3[:, :, :], t3[:, :, :], cp)
                if w0 >= 1024:
                    # post-preamble tail: Act has slack, DVE is the wall
                    nc.vector.tensor_scalar_max(t3[:, 0, :], t3[:, 0, :], 0.0)
                    nc.scalar.activation(t3[:, 1, :], t3[:, 1, :], AF.Relu)
                else:
                    nc.vector.tensor_scalar_max(t3[:, :, :], t3[:, :, :], 0.0)
                # W2T: out[span_block, 150] = t_block^T @ W2  (+ Z*b2)
                rT = kp.tile([128, nb, HID], b16, name="rT", tag=f"rT{w0}", bufs=3)
                for j in range(nb // 2):
                    ps = pw.tile([128, 2, HID], f32, name=f"ps_rT{j}", tag="psw")
                    for jj in range(2):
                        b = 2 * j + jj
                        bs = slice(b * 128, (b + 1) * 128)
                        for m in range(2):
                            nc.tensor.matmul(
                                ps[:, jj, :], t3[:, m, bs], w2h[m][:, :],
                                start=(m == 0), stop=(m == 1) and not use_b2)
                        if use_b2:
                            nc.tensor.matmul(ps[:, jj, :], zb[0:1, bs], b2r[:, :],
                                             start=False, stop=True)
                    nc.scalar.activation(rT[:, 2 * j:2 * j + 2, :],
                                         ps[:, :, :], AF.Relu)
                nc.sync.dma_start(
                    d_ro[n - 1, :, b0 * HID:(b0 + nb) * HID],
                    rT[:, :, :].rearrange("p b f -> p (b f)"))

        preamble(0, lead=True)
        preamble(1, lead=True)
        widthphase(0, 512)
        preamble(2)
        widthphase(512, 1024)
        preamble(3)
        widthphase(1024, 1536)
        preamble(4)
        widthphase(1536, 2048)
        widthphase(2048, 2560)
        nc.sync.dma_start(d_eo[0:1, :], e16[0:1, 0:SLAB])
    nc.compile()
    return nc


def _get_program(use_b2, use_b1):
    key = ("prog", use_b2, use_b1)
    if key not in _CACHE:
        _CACHE[key] = _build_program(use_b2, use_b1)
    return _CACHE[key]


def _prep_inputs(inputs):
    f32 = np.float32
    W = {k: np.asarray(v, f32) for k, v in inputs.items()}
    aw1 = W["attn_W1"].astype(BF16)   # [400, 150]
    w1a = W["sc_W1"][0:400].astype(BF16)
    w1b = W["sc_W1"][400:800].astype(BF16)
    w1c = W["sc_W1"][800:1100].astype(BF16)  # [300, 150]
    aw2 = W["attn_W2"].astype(BF16)
    aw3 = W["attn_W3"].astype(BF16)   # [150, 1]
    w2 = W["sc_W2"].astype(BF16)

    pk128 = np.zeros((128, 1650), BF16)
    for i in range(3):
        pk128[:, 150 * i:150 * (i + 1)] = aw1[128 * i:128 * (i + 1)]
        pk128[:, 450 + 150 * i:450 + 150 * (i + 1)] = w1a[128 * i:128 * (i + 1)]
        pk128[:, 900 + 150 * i:900 + 150 * (i + 1)] = w1b[128 * i:128 * (i + 1)]
    pk128[:, 1350:1500] = w1c[0:128]
    pk128[:, 1500:1650] = w1c[128:256]

    pk75 = np.zeros((HH, 752), BF16)
    pk75[:, 0:150] = aw2[0:75]
    pk75[:, 150:300] = aw2[75:150]
    pk75[:, 300:450] = w2[0:75]
    pk75[:, 450:600] = w2[75:150]
    pk75[:, 600:601] = aw3[0:75]
    pk75[:, 601:602] = aw3[75:150]
    pk75[0, 602:752] = W["sc_b2"].astype(BF16)

    pktail = np.zeros((44, 600), BF16)
    pktail[0:44, 0:150] = w1c[256:300]
    pktail[0:16, 150:300] = aw1[384:400]
    pktail[0:16, 300:450] = w1a[384:400]
    pktail[0:16, 450:600] = w1b[384:400]

    pkb = np.zeros((HH, 7), f32)
    pkb[:, 0] = W["attn_b1"][0:75]
    pkb[:, 1] = W["attn_b1"][75:150]
    pkb[:, 2] = W["attn_b2"][0:75]
    pkb[:, 3] = W["attn_b2"][75:150]
    pkb[:, 4] = W["sc_b1"][0:75]
    pkb[:, 5] = W["sc_b1"][75:150]
    pkb[0, 6] = W["attn_b3"][0]

    shared = {"pk128": pk128, "pk75": pk75, "pktail": pktail, "pkb": pkb}
    states, embeds = W["states"], W["embeds"]
    in_maps = []
    for d in range(NCORES):
        t0 = d * L
        hi = min(T, t0 + SLAB)
        ss = np.zeros((SLAB, SD), f32)
        se = np.zeros((SLAB, ED), f32)
        ss[:hi - t0] = states[t0:hi]
        se[:hi - t0] = embeds[t0:hi]
        m = dict(shared)
        m["sT"] = np.ascontiguousarray(ss.T).astype(BF16)
        m["eT"] = np.ascontiguousarray(se.T).astype(BF16)
        in_maps.append(m)
    return in_maps, float(W["sc_b3"][0])


_last_results = None


def kernel(**inputs):
    global _last_results
    from concourse.bass_utils import run_bass_kernel_spmd

    in_maps, b3 = _prep_inputs(inputs)
    use_b2 = bool(np.any(np.asarray(inputs["sc_b2"])))
    use_b1 = bool(np.any(np.asarray(inputs["sc_b1"])))
    nc = _get_program(use_b2, use_b1)
    res = run_bass_kernel_spmd(nc, in_maps, core_ids=list(range(NCORES)))
    _last_results = res
    outs = res.results
    w3 = np.asarray(inputs["sc_W3"], np.float32)[:, 0]  # [150]

    parts = []
    sp_all = []
    for d in range(NCORES):
        # r_out [NW, 128, 20*150] -> spn [NW, SLAB]
        r = np.asarray(outs[d]["r_out"])
        spn = (r.reshape(-1, HID).astype(np.float32) @ w3)
        spn = spn.reshape(NW, 128, SLAB // 128)
        spn = np.transpose(spn, (0, 2, 1)).reshape(NW, SLAB)
        sp_all.append(spn)
    for n in range(1, NW + 1):
        for d in range(NCORES):
            sp = sp_all[d][n - 1].astype(np.float64)
            e = np.asarray(outs[d]["e_out"])[0].astype(np.float64)
            csum = np.concatenate(([0.0], np.cumsum(e)))
            cnt = L if d < NCORES - 1 else L - (n - 1)
            z = csum[n:n + cnt] - csum[0:cnt]
            parts.append((sp[:cnt] / z + b3).astype(np.float32))
    return np.concatenate(parts)
